# revision 23
# baseline (speedup 1.0000x reference)
"""Complex-attention Bass/Tile kernel for Trainium2, sharded over 8 NeuronCores.

Problem (hardcoded): N=4, L=S=1024, H=8, D=64, fp32 inputs q/k/v (real+imag).
  s_r + i*s_i = (Qr + iQi)(Kr + iKi)^H-style complex QK^T (per reference),
  softmax(scale*s) independently on real/imag, complex AV, plus the
  head-mean of the attention weights as extra outputs.

Sharding: core c handles batch n = c//2 and query-rows l in [512*(c%2), +512).
Each core sees all 8 heads and the full key range, so outputs are pure
concatenations (the head-mean of the weights is complete per core).

On-chip layout: scores are computed transposed, S^T[s, l], with the complex
contraction packed into 128 partitions: lhsT = [k; -/+ k'] chunks of K^T,
rhs = [qr; qi]^T. exp on ScalarE (no max subtraction: inputs are unit-normal,
scaled scores ~ N(0,1), max < ~6). Row sums Z via ones-matmul on PE
(broadcast across partitions), 1/Z = exp(-ln Z) on ScalarE, weights
normalized in-place on VectorE against a PE-broadcast of 1/Z (reciprocal
via the fast custom-DVE op; no ACT table switching). AV accumulates
[u_r; u_i]^T in one PSUM bank with packed stationary operands
[Vr|Vi] / [-Vi|Vr], one full-array matmul per weight tile. The head-mean
`a` accumulates via identity-matmul PSUM accumulation in rounds of
3/3/2 heads; the two 3-head partials are stored bf16 and re-injected
into the final round's PSUM group by identity matmuls so the kernel
tail has no VectorE combine chain (evacuation rides the then-idle
ScalarE). Weight tiles for 4 heads stay resident in SBUF.
"""
import numpy as np
import ml_dtypes

try:
    import concourse.bass as bass  # noqa: F401
except ImportError:  # pragma: no cover - fallback for bare environments
    import sys
    for p in ("/root/.axon_site", "/root/.axon_site/_ro/trn_rl_repo",
              "/root/.axon_site/_ro/pypackages", "/opt/trn_rl_repo",
              "/opt/pypackages"):
        if p not in sys.path:
            sys.path.append(p)
    import concourse.bass as bass  # noqa: F401

import concourse.tile as tile
from concourse import bacc, mybir
from concourse.bass_utils import run_bass_kernel_spmd
from contextlib import ExitStack

F32 = mybir.dt.float32
BF16 = mybir.dt.bfloat16
AF = mybir.ActivationFunctionType
BF16NP = ml_dtypes.bfloat16

N, L, H, D = 4, 1024, 8, 64
LLOC = 512          # query rows per core
S = L               # key length
NCHUNK = S // 128   # 8 key chunks of 128
NPAIR = NCHUNK // 2
SCALE = 1.0 / np.sqrt(D).astype(np.float32)  # 0.125


def _emit(tc):
    nc = tc.nc
    qt = nc.dram_tensor("qt", [H, 128, LLOC], BF16, kind="ExternalInput").ap()
    ktr = nc.dram_tensor("ktr", [H, 128, S], BF16, kind="ExternalInput").ap()
    kti = nc.dram_tensor("kti", [H, 128, S], BF16, kind="ExternalInput").ap()
    va = nc.dram_tensor("va", [H, 128, NCHUNK * 128], BF16, kind="ExternalInput").ap()
    vb = nc.dram_tensor("vb", [H, 128, NCHUNK * 128], BF16, kind="ExternalInput").ap()
    ones_d = nc.dram_tensor("ones", [128, 128], BF16, kind="ExternalInput").ap()
    ident_d = nc.dram_tensor("ident", [128, 128], BF16, kind="ExternalInput").ap()

    ut = nc.dram_tensor("ut", [H, 128, LLOC], F32, kind="ExternalOutput").ap()
    at = nc.dram_tensor("at", [2, NCHUNK, 128, LLOC], BF16, kind="ExternalOutput").ap()

    with ExitStack() as ctx:
        const = ctx.enter_context(tc.tile_pool(name="const", bufs=1))
        kin = ctx.enter_context(tc.tile_pool(name="kin", bufs=1))
        wst = ctx.enter_context(tc.tile_pool(name="wst", bufs=8 * NPAIR))
        rows = ctx.enter_context(tc.tile_pool(name="rows", bufs=2))
        bcb = ctx.enter_context(tc.tile_pool(name="bcb", bufs=4))
        uev = ctx.enter_context(tc.tile_pool(name="uev", bufs=2))
        apart = ctx.enter_context(tc.tile_pool(name="apart", bufs=4))
        aev = ctx.enter_context(tc.tile_pool(name="aev", bufs=2))
        scr = ctx.enter_context(tc.tile_pool(name="scr", bufs=2, space="PSUM"))
        zb = ctx.enter_context(tc.tile_pool(name="zb", bufs=1, space="PSUM"))
        av = ctx.enter_context(tc.tile_pool(name="av", bufs=1, space="PSUM"))
        aps_p = ctx.enter_context(tc.tile_pool(name="aps_p", bufs=2, space="PSUM"))

        t_ones = const.tile([128, 128], BF16)
        t_ident = const.tile([128, 128], BF16)

        # resident inputs (per-head slices in free dim)
        t_qt = kin.tile([128, H * LLOC], BF16)
        t_ktr = kin.tile([128, H * S], BF16)
        t_kti = kin.tile([128, H * S], BF16)
        t_va = kin.tile([128, H * NCHUNK * 128], BF16)
        t_vb = kin.tile([128, H * NCHUNK * 128], BF16)
        # head-0 critical path first: q, then K-real in pieces (scores for
        # the real side can start after the first piece), ones (Z matmuls)
        nc.sync.dma_start(t_qt[:, 0:LLOC], qt[0])
        for j in range(4):
            nc.sync.dma_start(t_ktr[:, j * (S // 4):(j + 1) * (S // 4)],
                              ktr[0][:, j * (S // 4):(j + 1) * (S // 4)])
        nc.sync.dma_start(t_ones[:], ones_d[:])
        for j in range(4):
            nc.sync.dma_start(t_kti[:, j * (S // 4):(j + 1) * (S // 4)],
                              kti[0][:, j * (S // 4):(j + 1) * (S // 4)])
        nc.sync.dma_start(t_va[:, 0:NCHUNK * 128], va[0])
        nc.sync.dma_start(t_vb[:, 0:NCHUNK * 128], vb[0])
        nc.sync.dma_start(t_ident[:], ident_d[:])
        for h in range(1, H):
            nc.sync.dma_start(t_qt[:, h * LLOC:(h + 1) * LLOC], qt[h])
            nc.sync.dma_start(t_ktr[:, h * S:(h + 1) * S], ktr[h])
            nc.sync.dma_start(t_kti[:, h * S:(h + 1) * S], kti[h])
            nc.sync.dma_start(t_va[:, h * NCHUNK * 128:(h + 1) * NCHUNK * 128], va[h])
            nc.sync.dma_start(t_vb[:, h * NCHUNK * 128:(h + 1) * NCHUNK * 128], vb[h])

        w_tiles = {}       # (h % 4, ri, pair) -> [128, 1024] bf16 (chunk pair)
        a_part = {}        # ri -> list of [128, NCHUNK*LLOC] bf16 partials

        def w_slice(hm, ri, c):
            return w_tiles[(hm, ri, c // 2)][:, (c % 2) * LLOC:((c % 2) + 1) * LLOC]

        def a_round(heads, mode, idx=0, ris=(0, 1)):
            """Accumulate a^T over `heads` via identity matmuls.
            mode "partial": evacuate unscaled bf16 partial (slot idx).
            mode "final": re-inject bf16 partials via identity matmuls,
            evacuate scaled on ScalarE (idle at the kernel tail), DMA out."""
            for ri in ris:
                if mode == "partial" and idx == len(a_part.get(ri, ())):
                    a_part.setdefault(ri, []).append(apart.tile(
                        [128, NCHUNK * LLOC], BF16, tag="apart",
                        name=f"apart_{ri}_{idx}"))
                for c in range(NCHUNK):
                    aps = aps_p.tile([128, LLOC], F32, tag="aps")
                    nmm = len(heads) + (len(a_part.get(ri, ())) if mode == "final" else 0)
                    j = 0
                    for hh in heads:
                        nc.tensor.matmul(
                            aps[:], t_ident[:], w_slice(hh % 4, ri, c),
                            start=(j == 0), stop=(j == nmm - 1))
                        j += 1
                    if mode == "final":
                        for part in a_part[ri]:
                            nc.tensor.matmul(
                                aps[:], t_ident[:],
                                part[:, c * LLOC:(c + 1) * LLOC],
                                start=False, stop=(j == nmm - 1))
                            j += 1
                    if mode == "partial":
                        psl = a_part[ri][idx][:, c * LLOC:(c + 1) * LLOC]
                        nc.vector.tensor_copy(psl, aps[:])
                    else:
                        fin = aev.tile([128, LLOC], BF16, tag="aev")
                        nc.scalar.activation(fin[:], aps[:], AF.Copy,
                                             scale=1.0 / H)
                        nc.sync.dma_start(at[ri, c], fin[:])

        for h in range(H):
            hm = h % 4
            qs = t_qt[:, h * LLOC:(h + 1) * LLOC]
            ups = av.tile([128, LLOC], F32, tag="avp")
            vofs = h * NCHUNK * 128
            for ri in range(2):
                kt = t_ktr if ri == 0 else t_kti
                tv = t_va if ri == 0 else t_vb
                z_t = zb.tile([1, LLOC], F32, tag="zb", name=f"z_{ri}")
                for pair in range(NPAIR):
                    wp = wst.tile([128, 2 * LLOC], BF16, tag="wst", name=f"w_{hm}_{ri}_{pair}")
                    w_tiles[(hm, ri, pair)] = wp
                    sc = scr.tile([128, 2 * LLOC], F32, tag="scr")
                    for half in range(2):
                        c = 2 * pair + half
                        nc.tensor.matmul(
                            sc[:, half * LLOC:(half + 1) * LLOC],
                            kt[:, h * S + c * 128: h * S + (c + 1) * 128],
                            qs, start=True, stop=True)
                    # exp(scale * scores) -> bf16, unnormalized weights
                    nc.scalar.activation(wp[:], sc[:], AF.Exp, scale=float(SCALE))
                    # Z accumulation (row-sums over s, M=1)
                    for half in range(2):
                        nc.tensor.matmul(
                            z_t[:], t_ones[:, 0:1],
                            wp[:, half * LLOC:(half + 1) * LLOC],
                            start=(pair == 0 and half == 0),
                            stop=(pair == NPAIR - 1 and half == 1))
                # 1/Z, broadcast, normalize, and this side's AV accumulation
                rzf = rows.tile([1, LLOC], F32, tag="rzf")
                nc.vector.reciprocal_approx_fast(rzf[:], z_t[0:1, :])
                rz = rows.tile([1, LLOC], BF16, tag="rz")
                nc.vector.tensor_copy(rz[:], rzf[:])
                bc = bcb.tile([128, LLOC], BF16, tag="bcb", name=f"bc_{ri}")
                nc.gpsimd.partition_broadcast(bc[:], rz[:])
                for pair in range(NPAIR):
                    wp = w_tiles[(hm, ri, pair)]
                    # W = E * (1/Z), both chunks of the pair in one op
                    nc.vector.tensor_mul(
                        wp[:].rearrange("p (a b) -> p a b", a=2),
                        wp[:].rearrange("p (a b) -> p a b", a=2),
                        bc[:].unsqueeze(1).broadcast_to((128, 2, LLOC)))
                    for half in range(2):
                        c = 2 * pair + half
                        vsl = slice(vofs + c * 128, vofs + (c + 1) * 128)
                        nc.tensor.matmul(
                            ups[:], tv[:, vsl], w_slice(hm, ri, c),
                            start=(ri == 0 and c == 0),
                            stop=(ri == 1 and c == NCHUNK - 1))
                if h == 7:
                    # emit this side's final-round groups now: their W slices
                    # are ready, and the PE queue has slack during the other
                    # side's score/exp phase
                    a_round((4, 5, 6, 7), "final", ris=(ri,))
            u_sb = uev.tile([128, LLOC], F32, tag="uev")
            nc.vector.tensor_copy(u_sb[:], ups[:])
            nc.sync.dma_start(ut[h], u_sb[:])

            if h == 3:
                a_round((0, 1, 2, 3), "partial", 0)



_NC_CACHE = None


def _build_nc():
    global _NC_CACHE
    if _NC_CACHE is None:
        nc = bacc.Bacc("TRN2", target_bir_lowering=False, debug=False,
                       num_devices=8)
        with tile.TileContext(nc) as tc:
            _emit(tc)
        nc.compile()
        _NC_CACHE = nc
    return _NC_CACHE


def _host_prep(q_real, q_imag, k_real, k_imag, v_real, v_imag):
    """Build the 8 per-core input maps (host-side shard + transpose + cast)."""
    in_maps = []
    ones = np.ones((128, 128), dtype=BF16NP)
    ident = np.eye(128, dtype=np.float32).astype(BF16NP)
    per_n = {}
    for n in range(N):
        krT = np.ascontiguousarray(k_real[n].transpose(1, 2, 0))  # (H, D, S)
        kiT = np.ascontiguousarray(k_imag[n].transpose(1, 2, 0))
        ktr = np.concatenate([krT, -kiT], axis=1).astype(BF16NP)  # (H, 128, S)
        kti = np.concatenate([kiT, krT], axis=1).astype(BF16NP)
        # packed V: va = [Vr|Vi], vb = [-Vi|Vr], per chunk, (H, 128, NCHUNK*128)
        vrh = v_real[n].transpose(1, 0, 2).reshape(H, NCHUNK, 128, D)
        vih = v_imag[n].transpose(1, 0, 2).reshape(H, NCHUNK, 128, D)
        va_t = np.concatenate([vrh, vih], axis=3)          # (H, NCHUNK, 128, 128)
        vb_t = np.concatenate([-vih, vrh], axis=3)
        va_t = np.ascontiguousarray(va_t.transpose(0, 2, 1, 3)).reshape(
            H, 128, NCHUNK * 128).astype(BF16NP)
        vb_t = np.ascontiguousarray(vb_t.transpose(0, 2, 1, 3)).reshape(
            H, 128, NCHUNK * 128).astype(BF16NP)
        per_n[n] = (ktr, kti, va_t, vb_t)
    for core in range(8):
        n, half = divmod(core, 2)
        l0 = half * LLOC
        qrT = q_real[n, l0:l0 + LLOC].transpose(1, 2, 0)  # (H, D, LLOC)
        qiT = q_imag[n, l0:l0 + LLOC].transpose(1, 2, 0)
        qt = np.concatenate([qrT, qiT], axis=1).astype(BF16NP)  # (H, 128, LLOC)
        ktr, kti, va_t, vb_t = per_n[n]
        in_maps.append({
            "qt": np.ascontiguousarray(qt),
            "ktr": ktr, "kti": kti,
            "va": va_t, "vb": vb_t,
            "ones": ones, "ident": ident,
        })
    return in_maps


def _assemble(results):
    u_real = np.empty((N, L, H, D), dtype=np.float32)
    u_imag = np.empty((N, L, H, D), dtype=np.float32)
    a_real = np.empty((N, L, S), dtype=np.float32)
    a_imag = np.empty((N, L, S), dtype=np.float32)
    for core, r in enumerate(results):
        n, half = divmod(core, 2)
        l0 = half * LLOC
        ut = r["ut"]                     # (H, 128, LLOC)
        at = r["at"]                     # (2, NCHUNK, 128, LLOC)
        u_real[n, l0:l0 + LLOC] = ut[:, 0:64, :].transpose(2, 0, 1)
        u_imag[n, l0:l0 + LLOC] = ut[:, 64:128, :].transpose(2, 0, 1)
        a_real[n, l0:l0 + LLOC] = at[0].reshape(S, LLOC).T.astype(np.float32)
        a_imag[n, l0:l0 + LLOC] = at[1].reshape(S, LLOC).T.astype(np.float32)
    return u_real, u_imag, a_real, a_imag


def _run(inputs, trace=False, **kw):
    nc = _build_nc()
    in_maps = _host_prep(
        np.asarray(inputs["q_real"], dtype=np.float32),
        np.asarray(inputs["q_imag"], dtype=np.float32),
        np.asarray(inputs["k_real"], dtype=np.float32),
        np.asarray(inputs["k_imag"], dtype=np.float32),
        np.asarray(inputs["v_real"], dtype=np.float32),
        np.asarray(inputs["v_imag"], dtype=np.float32),
    )
    res = run_bass_kernel_spmd(nc, in_maps, list(range(8)), trace=trace, **kw)
    return res


def kernel(**inputs):
    res = _run(inputs, trace=False)
    return _assemble(res.results)


# revision 24
# speedup vs baseline: 1.0226x; 1.0226x over previous
"""Complex-attention Bass/Tile kernel for Trainium2, sharded over 8 NeuronCores.

Problem (hardcoded): N=4, L=S=1024, H=8, D=64, fp32 inputs q/k/v (real+imag).
  s_r + i*s_i = (Qr + iQi)(Kr + iKi)^H-style complex QK^T (per reference),
  softmax(scale*s) independently on real/imag, complex AV, plus the
  head-mean of the attention weights as extra outputs.

Sharding: core c handles batch n = c//2 and query-rows l in [512*(c%2), +512).
Each core sees all 8 heads and the full key range, so outputs are pure
concatenations (the head-mean of the weights is complete per core).

On-chip layout: scores are computed transposed, S^T[s, l], with the complex
contraction packed into 128 partitions: lhsT = [k; -/+ k'] chunks of K^T,
rhs = [qr; qi]^T. exp on ScalarE (no max subtraction: inputs are unit-normal,
scaled scores ~ N(0,1), max < ~6). Row sums Z via ones-matmul on PE
(broadcast across partitions), 1/Z = exp(-ln Z) on ScalarE, weights
normalized in-place on VectorE against a PE-broadcast of 1/Z (reciprocal
via the fast custom-DVE op; no ACT table switching). AV accumulates
[u_r; u_i]^T in one PSUM bank with packed stationary operands
[Vr|Vi] / [-Vi|Vr], one full-array matmul per weight tile. The head-mean
`a` accumulates via identity-matmul PSUM accumulation in rounds of
3/3/2 heads; the two 3-head partials are stored bf16 and re-injected
into the final round's PSUM group by identity matmuls so the kernel
tail has no VectorE combine chain (evacuation rides the then-idle
ScalarE). Weight tiles for 4 heads stay resident in SBUF.
"""
import numpy as np
import ml_dtypes

try:
    import concourse.bass as bass  # noqa: F401
except ImportError:  # pragma: no cover - fallback for bare environments
    import sys
    for p in ("/root/.axon_site", "/root/.axon_site/_ro/trn_rl_repo",
              "/root/.axon_site/_ro/pypackages", "/opt/trn_rl_repo",
              "/opt/pypackages"):
        if p not in sys.path:
            sys.path.append(p)
    import concourse.bass as bass  # noqa: F401

import concourse.tile as tile
from concourse import bacc, mybir
from concourse.bass_utils import run_bass_kernel_spmd
from contextlib import ExitStack

F32 = mybir.dt.float32
BF16 = mybir.dt.bfloat16
AF = mybir.ActivationFunctionType
BF16NP = ml_dtypes.bfloat16

N, L, H, D = 4, 1024, 8, 64
LLOC = 512          # query rows per core
S = L               # key length
NCHUNK = S // 128   # 8 key chunks of 128
NPAIR = NCHUNK // 2
SCALE = 1.0 / np.sqrt(D).astype(np.float32)  # 0.125


def _emit(tc):
    nc = tc.nc
    qt = nc.dram_tensor("qt", [H, 128, LLOC], BF16, kind="ExternalInput").ap()
    ktr = nc.dram_tensor("ktr", [H, 128, S], BF16, kind="ExternalInput").ap()
    kti = nc.dram_tensor("kti", [H, 128, S], BF16, kind="ExternalInput").ap()
    va = nc.dram_tensor("va", [H, 128, NCHUNK * 128], BF16, kind="ExternalInput").ap()
    vb = nc.dram_tensor("vb", [H, 128, NCHUNK * 128], BF16, kind="ExternalInput").ap()
    ones_d = nc.dram_tensor("ones", [128, 128], BF16, kind="ExternalInput").ap()
    ident_d = nc.dram_tensor("ident", [128, 128], BF16, kind="ExternalInput").ap()

    ut = nc.dram_tensor("ut", [H, 128, LLOC], F32, kind="ExternalOutput").ap()
    at = nc.dram_tensor("at", [2, NCHUNK, 128, LLOC], BF16, kind="ExternalOutput").ap()

    with ExitStack() as ctx:
        const = ctx.enter_context(tc.tile_pool(name="const", bufs=1))
        kin = ctx.enter_context(tc.tile_pool(name="kin", bufs=1))
        wst = ctx.enter_context(tc.tile_pool(name="wst", bufs=8 * NPAIR))
        rows = ctx.enter_context(tc.tile_pool(name="rows", bufs=2))
        bcb = ctx.enter_context(tc.tile_pool(name="bcb", bufs=4))
        uev = ctx.enter_context(tc.tile_pool(name="uev", bufs=2))
        apart = ctx.enter_context(tc.tile_pool(name="apart", bufs=4))
        aev = ctx.enter_context(tc.tile_pool(name="aev", bufs=2))
        scr = ctx.enter_context(tc.tile_pool(name="scr", bufs=2, space="PSUM"))
        zb = ctx.enter_context(tc.tile_pool(name="zb", bufs=1, space="PSUM"))
        av = ctx.enter_context(tc.tile_pool(name="av", bufs=1, space="PSUM"))
        aps_p = ctx.enter_context(tc.tile_pool(name="aps_p", bufs=2, space="PSUM"))

        t_ones = const.tile([128, 128], BF16)
        t_ident = const.tile([128, 128], BF16)

        # resident inputs (per-head slices in free dim)
        t_qt = kin.tile([128, H * LLOC], BF16)
        t_ktr = kin.tile([128, H * S], BF16)
        t_kti = kin.tile([128, H * S], BF16)
        t_va = kin.tile([128, H * NCHUNK * 128], BF16)
        t_vb = kin.tile([128, H * NCHUNK * 128], BF16)
        # head-0 critical path first: q, then K-real in pieces (scores for
        # the real side can start after the first piece), ones (Z matmuls)
        nc.sync.dma_start(t_qt[:, 0:LLOC], qt[0])
        for j in range(4):
            nc.sync.dma_start(t_ktr[:, j * (S // 4):(j + 1) * (S // 4)],
                              ktr[0][:, j * (S // 4):(j + 1) * (S // 4)])
        nc.sync.dma_start(t_ones[:], ones_d[:])
        for j in range(4):
            nc.sync.dma_start(t_kti[:, j * (S // 4):(j + 1) * (S // 4)],
                              kti[0][:, j * (S // 4):(j + 1) * (S // 4)])
        nc.sync.dma_start(t_va[:, 0:NCHUNK * 128], va[0])
        nc.sync.dma_start(t_vb[:, 0:NCHUNK * 128], vb[0])
        nc.sync.dma_start(t_ident[:], ident_d[:])
        for h in range(1, H):
            nc.sync.dma_start(t_qt[:, h * LLOC:(h + 1) * LLOC], qt[h])
            nc.sync.dma_start(t_ktr[:, h * S:(h + 1) * S], ktr[h])
            nc.sync.dma_start(t_kti[:, h * S:(h + 1) * S], kti[h])
            nc.sync.dma_start(t_va[:, h * NCHUNK * 128:(h + 1) * NCHUNK * 128], va[h])
            nc.sync.dma_start(t_vb[:, h * NCHUNK * 128:(h + 1) * NCHUNK * 128], vb[h])

        w_tiles = {}       # (h % 4, ri, pair) -> [128, 1024] bf16 (chunk pair)
        a_part = {}        # ri -> list of [128, NCHUNK*LLOC] bf16 partials

        def w_slice(hm, ri, c):
            return w_tiles[(hm, ri, c // 2)][:, (c % 2) * LLOC:((c % 2) + 1) * LLOC]

        def a_round(heads, mode, idx=0, ris=(0, 1)):
            """Accumulate a^T over `heads` via identity matmuls.
            mode "partial": evacuate unscaled bf16 partial (slot idx).
            mode "final": re-inject bf16 partials via identity matmuls,
            evacuate scaled on ScalarE (idle at the kernel tail), DMA out."""
            for ri in ris:
                if mode == "partial" and idx == len(a_part.get(ri, ())):
                    a_part.setdefault(ri, []).append(apart.tile(
                        [128, NCHUNK * LLOC], BF16, tag="apart",
                        name=f"apart_{ri}_{idx}"))
                for c in range(NCHUNK):
                    aps = aps_p.tile([128, LLOC], F32, tag="aps")
                    nmm = len(heads) + (len(a_part.get(ri, ())) if mode == "final" else 0)
                    j = 0
                    for hh in heads:
                        nc.tensor.matmul(
                            aps[:], t_ident[:], w_slice(hh % 4, ri, c),
                            start=(j == 0), stop=(j == nmm - 1))
                        j += 1
                    if mode == "final":
                        for part in a_part[ri]:
                            nc.tensor.matmul(
                                aps[:], t_ident[:],
                                part[:, c * LLOC:(c + 1) * LLOC],
                                start=False, stop=(j == nmm - 1))
                            j += 1
                    if mode == "partial":
                        psl = a_part[ri][idx][:, c * LLOC:(c + 1) * LLOC]
                        nc.vector.tensor_copy(psl, aps[:])
                    else:
                        fin = aev.tile([128, LLOC], BF16, tag="aev")
                        nc.scalar.activation(fin[:], aps[:], AF.Copy,
                                             scale=1.0 / H)
                        nc.sync.dma_start(at[ri, c], fin[:])

        for h in range(H):
            hm = h % 4
            qs = t_qt[:, h * LLOC:(h + 1) * LLOC]
            ups = av.tile([128, LLOC], F32, tag="avp")
            vofs = h * NCHUNK * 128
            for ri in range(2):
                kt = t_ktr if ri == 0 else t_kti
                tv = t_va if ri == 0 else t_vb
                z_t = zb.tile([1, LLOC], F32, tag="zb", name=f"z_{ri}")
                for pair in range(NPAIR):
                    wp = wst.tile([128, 2 * LLOC], BF16, tag="wst", name=f"w_{hm}_{ri}_{pair}")
                    w_tiles[(hm, ri, pair)] = wp
                    sc = scr.tile([128, 2 * LLOC], F32, tag="scr")
                    for half in range(2):
                        c = 2 * pair + half
                        nc.tensor.matmul(
                            sc[:, half * LLOC:(half + 1) * LLOC],
                            kt[:, h * S + c * 128: h * S + (c + 1) * 128],
                            qs, start=True, stop=True)
                    # exp(scale * scores) -> bf16, unnormalized weights
                    nc.scalar.activation(wp[:], sc[:], AF.Exp, scale=float(SCALE))
                    # Z accumulation (row-sums over s, M=1)
                    for half in range(2):
                        nc.tensor.matmul(
                            z_t[:], t_ones[:, 0:1],
                            wp[:, half * LLOC:(half + 1) * LLOC],
                            start=(pair == 0 and half == 0),
                            stop=(pair == NPAIR - 1 and half == 1))
                # 1/Z, broadcast, normalize, and this side's AV accumulation
                rzf = rows.tile([1, LLOC], F32, tag="rzf")
                nc.vector.reciprocal_approx_fast(rzf[:], z_t[0:1, :])
                rz = rows.tile([1, LLOC], BF16, tag="rz")
                nc.vector.tensor_copy(rz[:], rzf[:])
                bc_ps = zb.tile([128, LLOC], F32, tag="zb")
                nc.tensor.matmul(bc_ps[:], t_ones[0:1, :], rz[:],
                                 start=True, stop=True)
                bc = bcb.tile([128, LLOC], BF16, tag="bcb", name=f"bc_{ri}")
                nc.vector.tensor_copy(bc[:], bc_ps[:])
                for pair in range(NPAIR):
                    wp = w_tiles[(hm, ri, pair)]
                    # W = E * (1/Z), both chunks of the pair in one op
                    nc.vector.tensor_mul(
                        wp[:].rearrange("p (a b) -> p a b", a=2),
                        wp[:].rearrange("p (a b) -> p a b", a=2),
                        bc[:].unsqueeze(1).broadcast_to((128, 2, LLOC)))
                    for half in range(2):
                        c = 2 * pair + half
                        vsl = slice(vofs + c * 128, vofs + (c + 1) * 128)
                        nc.tensor.matmul(
                            ups[:], tv[:, vsl], w_slice(hm, ri, c),
                            start=(ri == 0 and c == 0),
                            stop=(ri == 1 and c == NCHUNK - 1))
                if h == 7:
                    # emit this side's final-round groups now: their W slices
                    # are ready, and the PE queue has slack during the other
                    # side's score/exp phase
                    a_round((4, 5, 6, 7), "final", ris=(ri,))
            u_sb = uev.tile([128, LLOC], F32, tag="uev")
            nc.vector.tensor_copy(u_sb[:], ups[:])
            nc.sync.dma_start(ut[h], u_sb[:])

            if h == 3:
                a_round((0, 1, 2, 3), "partial", 0)



_NC_CACHE = None


def _build_nc():
    global _NC_CACHE
    if _NC_CACHE is None:
        nc = bacc.Bacc("TRN2", target_bir_lowering=False, debug=False,
                       num_devices=8)
        with tile.TileContext(nc) as tc:
            _emit(tc)
        nc.compile()
        _NC_CACHE = nc
    return _NC_CACHE


def _host_prep(q_real, q_imag, k_real, k_imag, v_real, v_imag):
    """Build the 8 per-core input maps (host-side shard + transpose + cast)."""
    in_maps = []
    ones = np.ones((128, 128), dtype=BF16NP)
    ident = np.eye(128, dtype=np.float32).astype(BF16NP)
    per_n = {}
    for n in range(N):
        krT = np.ascontiguousarray(k_real[n].transpose(1, 2, 0))  # (H, D, S)
        kiT = np.ascontiguousarray(k_imag[n].transpose(1, 2, 0))
        ktr = np.concatenate([krT, -kiT], axis=1).astype(BF16NP)  # (H, 128, S)
        kti = np.concatenate([kiT, krT], axis=1).astype(BF16NP)
        # packed V: va = [Vr|Vi], vb = [-Vi|Vr], per chunk, (H, 128, NCHUNK*128)
        vrh = v_real[n].transpose(1, 0, 2).reshape(H, NCHUNK, 128, D)
        vih = v_imag[n].transpose(1, 0, 2).reshape(H, NCHUNK, 128, D)
        va_t = np.concatenate([vrh, vih], axis=3)          # (H, NCHUNK, 128, 128)
        vb_t = np.concatenate([-vih, vrh], axis=3)
        va_t = np.ascontiguousarray(va_t.transpose(0, 2, 1, 3)).reshape(
            H, 128, NCHUNK * 128).astype(BF16NP)
        vb_t = np.ascontiguousarray(vb_t.transpose(0, 2, 1, 3)).reshape(
            H, 128, NCHUNK * 128).astype(BF16NP)
        per_n[n] = (ktr, kti, va_t, vb_t)
    for core in range(8):
        n, half = divmod(core, 2)
        l0 = half * LLOC
        qrT = q_real[n, l0:l0 + LLOC].transpose(1, 2, 0)  # (H, D, LLOC)
        qiT = q_imag[n, l0:l0 + LLOC].transpose(1, 2, 0)
        qt = np.concatenate([qrT, qiT], axis=1).astype(BF16NP)  # (H, 128, LLOC)
        ktr, kti, va_t, vb_t = per_n[n]
        in_maps.append({
            "qt": np.ascontiguousarray(qt),
            "ktr": ktr, "kti": kti,
            "va": va_t, "vb": vb_t,
            "ones": ones, "ident": ident,
        })
    return in_maps


def _assemble(results):
    u_real = np.empty((N, L, H, D), dtype=np.float32)
    u_imag = np.empty((N, L, H, D), dtype=np.float32)
    a_real = np.empty((N, L, S), dtype=np.float32)
    a_imag = np.empty((N, L, S), dtype=np.float32)
    for core, r in enumerate(results):
        n, half = divmod(core, 2)
        l0 = half * LLOC
        ut = r["ut"]                     # (H, 128, LLOC)
        at = r["at"]                     # (2, NCHUNK, 128, LLOC)
        u_real[n, l0:l0 + LLOC] = ut[:, 0:64, :].transpose(2, 0, 1)
        u_imag[n, l0:l0 + LLOC] = ut[:, 64:128, :].transpose(2, 0, 1)
        a_real[n, l0:l0 + LLOC] = at[0].reshape(S, LLOC).T.astype(np.float32)
        a_imag[n, l0:l0 + LLOC] = at[1].reshape(S, LLOC).T.astype(np.float32)
    return u_real, u_imag, a_real, a_imag


def _run(inputs, trace=False, **kw):
    nc = _build_nc()
    in_maps = _host_prep(
        np.asarray(inputs["q_real"], dtype=np.float32),
        np.asarray(inputs["q_imag"], dtype=np.float32),
        np.asarray(inputs["k_real"], dtype=np.float32),
        np.asarray(inputs["k_imag"], dtype=np.float32),
        np.asarray(inputs["v_real"], dtype=np.float32),
        np.asarray(inputs["v_imag"], dtype=np.float32),
    )
    res = run_bass_kernel_spmd(nc, in_maps, list(range(8)), trace=trace, **kw)
    return res


def kernel(**inputs):
    res = _run(inputs, trace=False)
    return _assemble(res.results)


# revision 25
# speedup vs baseline: 1.0352x; 1.0123x over previous
"""Complex-attention Bass/Tile kernel for Trainium2, sharded over 8 NeuronCores.

Problem (hardcoded): N=4, L=S=1024, H=8, D=64, fp32 inputs q/k/v (real+imag).
  s_r + i*s_i = (Qr + iQi)(Kr + iKi)^H-style complex QK^T (per reference),
  softmax(scale*s) independently on real/imag, complex AV, plus the
  head-mean of the attention weights as extra outputs.

Sharding: core c handles batch n = c//2 and query-rows l in [512*(c%2), +512).
Each core sees all 8 heads and the full key range, so outputs are pure
concatenations (the head-mean of the weights is complete per core).

On-chip layout: scores are computed transposed, S^T[s, l], with the complex
contraction packed into 128 partitions: lhsT = [k; -/+ k'] chunks of K^T,
rhs = [qr; qi]^T. exp on ScalarE (no max subtraction: inputs are unit-normal,
scaled scores ~ N(0,1), max < ~6). Row sums Z via ones-matmul on PE
(broadcast across partitions), 1/Z = exp(-ln Z) on ScalarE, weights
normalized in-place on VectorE against a PE-broadcast of 1/Z (reciprocal
via the fast custom-DVE op; no ACT table switching). AV accumulates
[u_r; u_i]^T in one PSUM bank with packed stationary operands
[Vr|Vi] / [-Vi|Vr], one full-array matmul per weight tile. The head-mean
`a` accumulates via identity-matmul PSUM accumulation in rounds of
3/3/2 heads; the two 3-head partials are stored bf16 and re-injected
into the final round's PSUM group by identity matmuls so the kernel
tail has no VectorE combine chain (evacuation rides the then-idle
ScalarE). Weight tiles for 4 heads stay resident in SBUF.
"""
import numpy as np
import ml_dtypes

try:
    import concourse.bass as bass  # noqa: F401
except ImportError:  # pragma: no cover - fallback for bare environments
    import sys
    for p in ("/root/.axon_site", "/root/.axon_site/_ro/trn_rl_repo",
              "/root/.axon_site/_ro/pypackages", "/opt/trn_rl_repo",
              "/opt/pypackages"):
        if p not in sys.path:
            sys.path.append(p)
    import concourse.bass as bass  # noqa: F401

import concourse.tile as tile
from concourse import bacc, mybir
from concourse.bass_utils import run_bass_kernel_spmd
from contextlib import ExitStack

F32 = mybir.dt.float32
BF16 = mybir.dt.bfloat16
AF = mybir.ActivationFunctionType
BF16NP = ml_dtypes.bfloat16

N, L, H, D = 4, 1024, 8, 64
LLOC = 512          # query rows per core
S = L               # key length
NCHUNK = S // 128   # 8 key chunks of 128
NPAIR = NCHUNK // 2
SCALE = 1.0 / np.sqrt(D).astype(np.float32)  # 0.125


def _emit(tc):
    nc = tc.nc
    qt = nc.dram_tensor("qt", [H, 128, LLOC], BF16, kind="ExternalInput").ap()
    ktr = nc.dram_tensor("ktr", [H, 128, S], BF16, kind="ExternalInput").ap()
    kti = nc.dram_tensor("kti", [H, 128, S], BF16, kind="ExternalInput").ap()
    va = nc.dram_tensor("va", [H, 128, NCHUNK * 128], BF16, kind="ExternalInput").ap()
    vb = nc.dram_tensor("vb", [H, 128, NCHUNK * 128], BF16, kind="ExternalInput").ap()
    ones_d = nc.dram_tensor("ones", [128, 128], BF16, kind="ExternalInput").ap()
    ident_d = nc.dram_tensor("ident", [128, 128], BF16, kind="ExternalInput").ap()

    ut = nc.dram_tensor("ut", [H, 128, LLOC], F32, kind="ExternalOutput").ap()
    at = nc.dram_tensor("at", [2, NCHUNK, 128, LLOC], BF16, kind="ExternalOutput").ap()

    with ExitStack() as ctx:
        const = ctx.enter_context(tc.tile_pool(name="const", bufs=1))
        kin = ctx.enter_context(tc.tile_pool(name="kin", bufs=1))
        wst = ctx.enter_context(tc.tile_pool(name="wst", bufs=8 * NPAIR))
        rows = ctx.enter_context(tc.tile_pool(name="rows", bufs=2))
        bcb = ctx.enter_context(tc.tile_pool(name="bcb", bufs=4))
        uev = ctx.enter_context(tc.tile_pool(name="uev", bufs=2))
        apart = ctx.enter_context(tc.tile_pool(name="apart", bufs=4))
        aev = ctx.enter_context(tc.tile_pool(name="aev", bufs=2))
        scr = ctx.enter_context(tc.tile_pool(name="scr", bufs=2, space="PSUM"))
        zb = ctx.enter_context(tc.tile_pool(name="zb", bufs=1, space="PSUM"))
        av = ctx.enter_context(tc.tile_pool(name="av", bufs=1, space="PSUM"))
        aps_p = ctx.enter_context(tc.tile_pool(name="aps_p", bufs=2, space="PSUM"))

        t_ones = const.tile([128, 128], BF16)
        t_ident = const.tile([128, 128], BF16)

        # Warmup during the preamble/DMA-wait window (no input deps):
        # ~56 tiny matmuls keep the PE HAM monitor busy so the real stream
        # starts at 2.4 GHz, and one dummy exp pulls the ACT table load off
        # head 0's critical path.
        t_warm = const.tile([128, 64], BF16)
        nc.vector.memset(t_warm[:], 1.0)
        w_out = const.tile([1, 64], BF16)
        nc.scalar.activation(w_out[:], t_warm[0:1, :], AF.Exp)
        wp_ps = scr.tile([128, 2 * LLOC], F32, tag="scr")
        for _ in range(56):
            nc.tensor.matmul(wp_ps[0:64, 0:64], t_warm[:], t_warm[:],
                             start=True, stop=True)

        # resident inputs (per-head slices in free dim)
        t_qt = kin.tile([128, H * LLOC], BF16)
        t_ktr = kin.tile([128, H * S], BF16)
        t_kti = kin.tile([128, H * S], BF16)
        t_va = kin.tile([128, H * NCHUNK * 128], BF16)
        t_vb = kin.tile([128, H * NCHUNK * 128], BF16)
        # head-0 critical path first: q, then K-real in pieces (scores for
        # the real side can start after the first piece), ones (Z matmuls)
        nc.sync.dma_start(t_qt[:, 0:LLOC], qt[0])
        for j in range(4):
            nc.sync.dma_start(t_ktr[:, j * (S // 4):(j + 1) * (S // 4)],
                              ktr[0][:, j * (S // 4):(j + 1) * (S // 4)])
        nc.sync.dma_start(t_ones[:], ones_d[:])
        for j in range(4):
            nc.sync.dma_start(t_kti[:, j * (S // 4):(j + 1) * (S // 4)],
                              kti[0][:, j * (S // 4):(j + 1) * (S // 4)])
        nc.sync.dma_start(t_va[:, 0:NCHUNK * 128], va[0])
        nc.sync.dma_start(t_vb[:, 0:NCHUNK * 128], vb[0])
        nc.sync.dma_start(t_ident[:], ident_d[:])
        for h in range(1, H):
            nc.sync.dma_start(t_qt[:, h * LLOC:(h + 1) * LLOC], qt[h])
            nc.sync.dma_start(t_ktr[:, h * S:(h + 1) * S], ktr[h])
            nc.sync.dma_start(t_kti[:, h * S:(h + 1) * S], kti[h])
            nc.sync.dma_start(t_va[:, h * NCHUNK * 128:(h + 1) * NCHUNK * 128], va[h])
            nc.sync.dma_start(t_vb[:, h * NCHUNK * 128:(h + 1) * NCHUNK * 128], vb[h])

        w_tiles = {}       # (h % 4, ri, pair) -> [128, 1024] bf16 (chunk pair)
        a_part = {}        # ri -> list of [128, NCHUNK*LLOC] bf16 partials

        def w_slice(hm, ri, c):
            return w_tiles[(hm, ri, c // 2)][:, (c % 2) * LLOC:((c % 2) + 1) * LLOC]

        def a_round(heads, mode, idx=0, ris=(0, 1)):
            """Accumulate a^T over `heads` via identity matmuls.
            mode "partial": evacuate unscaled bf16 partial (slot idx).
            mode "final": re-inject bf16 partials via identity matmuls,
            evacuate scaled on ScalarE (idle at the kernel tail), DMA out."""
            for ri in ris:
                if mode == "partial" and idx == len(a_part.get(ri, ())):
                    a_part.setdefault(ri, []).append(apart.tile(
                        [128, NCHUNK * LLOC], BF16, tag="apart",
                        name=f"apart_{ri}_{idx}"))
                for c in range(NCHUNK):
                    aps = aps_p.tile([128, LLOC], F32, tag="aps")
                    nmm = len(heads) + (len(a_part.get(ri, ())) if mode == "final" else 0)
                    j = 0
                    for hh in heads:
                        nc.tensor.matmul(
                            aps[:], t_ident[:], w_slice(hh % 4, ri, c),
                            start=(j == 0), stop=(j == nmm - 1))
                        j += 1
                    if mode == "final":
                        for part in a_part[ri]:
                            nc.tensor.matmul(
                                aps[:], t_ident[:],
                                part[:, c * LLOC:(c + 1) * LLOC],
                                start=False, stop=(j == nmm - 1))
                            j += 1
                    if mode == "partial":
                        psl = a_part[ri][idx][:, c * LLOC:(c + 1) * LLOC]
                        nc.vector.tensor_copy(psl, aps[:])
                    else:
                        fin = aev.tile([128, LLOC], BF16, tag="aev")
                        nc.scalar.activation(fin[:], aps[:], AF.Copy,
                                             scale=1.0 / H)
                        nc.sync.dma_start(at[ri, c], fin[:])

        for h in range(H):
            hm = h % 4
            qs = t_qt[:, h * LLOC:(h + 1) * LLOC]
            ups = av.tile([128, LLOC], F32, tag="avp")
            vofs = h * NCHUNK * 128
            for ri in range(2):
                kt = t_ktr if ri == 0 else t_kti
                tv = t_va if ri == 0 else t_vb
                z_t = zb.tile([1, LLOC], F32, tag="zb", name=f"z_{ri}")
                for pair in range(NPAIR):
                    wp = wst.tile([128, 2 * LLOC], BF16, tag="wst", name=f"w_{hm}_{ri}_{pair}")
                    w_tiles[(hm, ri, pair)] = wp
                    sc = scr.tile([128, 2 * LLOC], F32, tag="scr")
                    for half in range(2):
                        c = 2 * pair + half
                        nc.tensor.matmul(
                            sc[:, half * LLOC:(half + 1) * LLOC],
                            kt[:, h * S + c * 128: h * S + (c + 1) * 128],
                            qs, start=True, stop=True)
                    # exp(scale * scores) -> bf16, unnormalized weights
                    nc.scalar.activation(wp[:], sc[:], AF.Exp, scale=float(SCALE))
                    # Z accumulation (row-sums over s, M=1)
                    for half in range(2):
                        nc.tensor.matmul(
                            z_t[:], t_ones[:, 0:1],
                            wp[:, half * LLOC:(half + 1) * LLOC],
                            start=(pair == 0 and half == 0),
                            stop=(pair == NPAIR - 1 and half == 1))
                # 1/Z, broadcast, normalize, and this side's AV accumulation
                rzf = rows.tile([1, LLOC], F32, tag="rzf")
                nc.vector.reciprocal_approx_fast(rzf[:], z_t[0:1, :])
                rz = rows.tile([1, LLOC], BF16, tag="rz")
                nc.vector.tensor_copy(rz[:], rzf[:])
                bc_ps = zb.tile([128, LLOC], F32, tag="zb")
                nc.tensor.matmul(bc_ps[:], t_ones[0:1, :], rz[:],
                                 start=True, stop=True)
                bc = bcb.tile([128, LLOC], BF16, tag="bcb", name=f"bc_{ri}")
                nc.vector.tensor_copy(bc[:], bc_ps[:])
                for pair in range(NPAIR):
                    wp = w_tiles[(hm, ri, pair)]
                    # W = E * (1/Z), both chunks of the pair in one op
                    nc.vector.tensor_mul(
                        wp[:].rearrange("p (a b) -> p a b", a=2),
                        wp[:].rearrange("p (a b) -> p a b", a=2),
                        bc[:].unsqueeze(1).broadcast_to((128, 2, LLOC)))
                    for half in range(2):
                        c = 2 * pair + half
                        vsl = slice(vofs + c * 128, vofs + (c + 1) * 128)
                        nc.tensor.matmul(
                            ups[:], tv[:, vsl], w_slice(hm, ri, c),
                            start=(ri == 0 and c == 0),
                            stop=(ri == 1 and c == NCHUNK - 1))
                if h == 7:
                    # emit this side's final-round groups now: their W slices
                    # are ready, and the PE queue has slack during the other
                    # side's score/exp phase
                    a_round((4, 5, 6, 7), "final", ris=(ri,))
            u_sb = uev.tile([128, LLOC], F32, tag="uev")
            nc.vector.tensor_copy(u_sb[:], ups[:])
            nc.sync.dma_start(ut[h], u_sb[:])

            if h == 3:
                a_round((0, 1, 2, 3), "partial", 0)



_NC_CACHE = None


def _build_nc():
    global _NC_CACHE
    if _NC_CACHE is None:
        nc = bacc.Bacc("TRN2", target_bir_lowering=False, debug=False,
                       num_devices=8)
        with tile.TileContext(nc) as tc:
            _emit(tc)
        nc.compile()
        _NC_CACHE = nc
    return _NC_CACHE


def _host_prep(q_real, q_imag, k_real, k_imag, v_real, v_imag):
    """Build the 8 per-core input maps (host-side shard + transpose + cast)."""
    in_maps = []
    ones = np.ones((128, 128), dtype=BF16NP)
    ident = np.eye(128, dtype=np.float32).astype(BF16NP)
    per_n = {}
    for n in range(N):
        krT = np.ascontiguousarray(k_real[n].transpose(1, 2, 0))  # (H, D, S)
        kiT = np.ascontiguousarray(k_imag[n].transpose(1, 2, 0))
        ktr = np.concatenate([krT, -kiT], axis=1).astype(BF16NP)  # (H, 128, S)
        kti = np.concatenate([kiT, krT], axis=1).astype(BF16NP)
        # packed V: va = [Vr|Vi], vb = [-Vi|Vr], per chunk, (H, 128, NCHUNK*128)
        vrh = v_real[n].transpose(1, 0, 2).reshape(H, NCHUNK, 128, D)
        vih = v_imag[n].transpose(1, 0, 2).reshape(H, NCHUNK, 128, D)
        va_t = np.concatenate([vrh, vih], axis=3)          # (H, NCHUNK, 128, 128)
        vb_t = np.concatenate([-vih, vrh], axis=3)
        va_t = np.ascontiguousarray(va_t.transpose(0, 2, 1, 3)).reshape(
            H, 128, NCHUNK * 128).astype(BF16NP)
        vb_t = np.ascontiguousarray(vb_t.transpose(0, 2, 1, 3)).reshape(
            H, 128, NCHUNK * 128).astype(BF16NP)
        per_n[n] = (ktr, kti, va_t, vb_t)
    for core in range(8):
        n, half = divmod(core, 2)
        l0 = half * LLOC
        qrT = q_real[n, l0:l0 + LLOC].transpose(1, 2, 0)  # (H, D, LLOC)
        qiT = q_imag[n, l0:l0 + LLOC].transpose(1, 2, 0)
        qt = np.concatenate([qrT, qiT], axis=1).astype(BF16NP)  # (H, 128, LLOC)
        ktr, kti, va_t, vb_t = per_n[n]
        in_maps.append({
            "qt": np.ascontiguousarray(qt),
            "ktr": ktr, "kti": kti,
            "va": va_t, "vb": vb_t,
            "ones": ones, "ident": ident,
        })
    return in_maps


def _assemble(results):
    u_real = np.empty((N, L, H, D), dtype=np.float32)
    u_imag = np.empty((N, L, H, D), dtype=np.float32)
    a_real = np.empty((N, L, S), dtype=np.float32)
    a_imag = np.empty((N, L, S), dtype=np.float32)
    for core, r in enumerate(results):
        n, half = divmod(core, 2)
        l0 = half * LLOC
        ut = r["ut"]                     # (H, 128, LLOC)
        at = r["at"]                     # (2, NCHUNK, 128, LLOC)
        u_real[n, l0:l0 + LLOC] = ut[:, 0:64, :].transpose(2, 0, 1)
        u_imag[n, l0:l0 + LLOC] = ut[:, 64:128, :].transpose(2, 0, 1)
        a_real[n, l0:l0 + LLOC] = at[0].reshape(S, LLOC).T.astype(np.float32)
        a_imag[n, l0:l0 + LLOC] = at[1].reshape(S, LLOC).T.astype(np.float32)
    return u_real, u_imag, a_real, a_imag


def _run(inputs, trace=False, **kw):
    nc = _build_nc()
    in_maps = _host_prep(
        np.asarray(inputs["q_real"], dtype=np.float32),
        np.asarray(inputs["q_imag"], dtype=np.float32),
        np.asarray(inputs["k_real"], dtype=np.float32),
        np.asarray(inputs["k_imag"], dtype=np.float32),
        np.asarray(inputs["v_real"], dtype=np.float32),
        np.asarray(inputs["v_imag"], dtype=np.float32),
    )
    res = run_bass_kernel_spmd(nc, in_maps, list(range(8)), trace=trace, **kw)
    return res


def kernel(**inputs):
    res = _run(inputs, trace=False)
    return _assemble(res.results)
